# revision 4
# baseline (speedup 1.0000x reference)
"""HaarConv2D (depthwise 2x2 stride-2 Haar transform) on 8 Trainium2 cores.

Input  x: [16, 64, 512, 512] f32
Output (low_pass, detail): each [16, 64, 256, 256] f32
  low = 0.5*(a+b+c+d),  det = 0.5*(a-b-c+d)  over each non-overlapping
  2x2 block, where a,b,c,d are the TL/TR/BL/BR elements.

Sharding: pure data parallel over batch — core i handles batches [2i, 2i+1].
Per-core layout: SBUF partition p = (b_local*64 + channel) image plane
(128 planes of 512x512); free dim = image rows.

Memory-bound problem. Per-core HBM traffic is fixed at 192 MiB (128 in +
64 out); single-core roofline 563 us at 358 GB/s, but with all 8 cores
running the chip-level HBM wall makes the per-core floor ~600-650 us
(aggregate ~2.4 TB/s measured). This kernel measures ~640-750 us/invocation
steady state (amortized repeat-in-NEFF), statistically tied with every
deeper-buffered variant tried — the wall is chip HBM bandwidth, not
structure. Chosen variant minimizes instruction count (~112) and DMA count
(32) at equal steady-state speed:

  16 iterations; each loads 32 rows/plane (8 MiB DMA, 64 KiB/partition
  contiguous, double-buffered), computes 16 output rows/plane, stores
  low+det packed as one 4 MiB DMA (out[P, 2, 256, 256], ch0=low, ch1=det).
  DVE, in place in the output tile (identical-AP in-place ops are race-safe):
    p = a + d ; q = b + c      (strided tensor_tensor)
    p <- p + q                 (= 2*low)
    q <- (q * -2) + p          (scalar_tensor_tensor; = 2*det)
  ACT: one merged uv *= 0.5 over both channels.
  All DMAs on the sync (SP) HWDGE ring — measured: output DMAs on the ACT
  ring are ~1.8x worse; direct input->output DMA (no compute decoupling)
  ~1.5x worse; R=8 with 4-deep input prefetch statistically tied.
"""

import numpy as np

import concourse.bacc as bacc
import concourse.mybir as mybir
import concourse.tile as tile
from concourse.bass_utils import run_bass_kernel_spmd

B, C, H, W = 16, 64, 512, 512
NCORES = 8
BPC = B // NCORES            # batches per core
P = BPC * C                  # 128 planes per core = SBUF partitions
R = 16                       # output rows per plane per iteration
ITERS = (H // 2) // R        # 16
F32 = mybir.dt.float32

TRACE = False                # test.py may set this
LAST_RESULTS = None          # BassKernelResults of the last run (for test.py)

_nc = None


def _build():
    nc = bacc.Bacc("TRN2", target_bir_lowering=False, debug=False)
    x = nc.dram_tensor("x", [P, H, W], F32, kind="ExternalInput")
    out = nc.dram_tensor("out", [P, 2, H // 2, W // 2], F32,
                         kind="ExternalOutput")

    with tile.TileContext(nc) as tc:
        with (
            tc.tile_pool(name="inp", bufs=2) as inp,
            tc.tile_pool(name="pq", bufs=2) as pqp,
        ):
            for i in range(ITERS):
                t = inp.tile([P, 2 * R, W], F32, tag="t")
                nc.sync.dma_start(out=t[:], in_=x[:, 2 * R * i:2 * R * (i + 1), :])
                a = t[:, 0:2 * R:2, 0:W:2]
                b = t[:, 0:2 * R:2, 1:W:2]
                c = t[:, 1:2 * R:2, 0:W:2]
                d = t[:, 1:2 * R:2, 1:W:2]
                uv = pqp.tile([P, 2, R, W // 2], F32, tag="uv")
                p = uv[:, 0]
                q = uv[:, 1]
                nc.vector.tensor_tensor(out=p, in0=a, in1=d,
                                        op=mybir.AluOpType.add)
                nc.vector.tensor_tensor(out=q, in0=b, in1=c,
                                        op=mybir.AluOpType.add)
                nc.vector.tensor_tensor(out=p, in0=p, in1=q,
                                        op=mybir.AluOpType.add)
                nc.vector.scalar_tensor_tensor(
                    out=q, in0=q, scalar=-2.0, in1=p,
                    op0=mybir.AluOpType.mult, op1=mybir.AluOpType.add)
                nc.scalar.mul(out=uv[:], in_=uv[:], mul=0.5)
                nc.sync.dma_start(out=out[:, :, R * i:R * (i + 1), :],
                                  in_=uv[:])
    nc.compile()
    return nc


def _get_nc():
    global _nc
    if _nc is None:
        _nc = _build()
    return _nc


def kernel(x):
    global LAST_RESULTS
    x = np.ascontiguousarray(np.asarray(x), dtype=np.float32)
    assert x.shape == (B, C, H, W), x.shape
    nc = _get_nc()
    in_maps = [
        {"x": x[i * BPC:(i + 1) * BPC].reshape(P, H, W)} for i in range(NCORES)
    ]
    last_err = None
    for _attempt in range(3):
        try:
            res = run_bass_kernel_spmd(nc, in_maps, list(range(NCORES)),
                                       trace=TRACE)
            break
        except Exception as e:  # transient NRT device errors happen; retry
            last_err = e
    else:
        raise last_err
    LAST_RESULTS = res
    low = np.concatenate(
        [r["out"][:, 0].reshape(BPC, C, H // 2, W // 2) for r in res.results],
        axis=0)
    det = np.concatenate(
        [r["out"][:, 1].reshape(BPC, C, H // 2, W // 2) for r in res.results],
        axis=0)
    return (low, det)
